# revision 3
# baseline (speedup 1.0000x reference)
"""Trainium2 Bass kernel for the 3-body Hamiltonian-NN time-derivative.

out = J grad_z H(z): dqdt = p * minv (trivial affine map, assembled on the
host); dpdt = pairwise forces, computed on device from q only.

The potential's tiny MLP acts on the scalar pairwise inverse distance
s = 1/sqrt(r2+eps2), so its gradient contribution reduces to a smooth 1-D
coefficient C(s) = g(s)*s^3 multiplying each pair-difference vector.  C is
distilled at runtime from the MLP weights into a degree-5 polynomial in s
(Chebyshev fit on s in [1/sqrt(130), 10]; end-to-end rel err 9.1e-3 vs the
2e-2 gate, f16 Horner included).

Device pipeline per 128-row chunk (2 chunks of T=128 per core, pure data
parallel over 8 cores):
  DVE : pair diffs (f32, keeps cancellation digits), r2 eps-fold STT,
        5-step Horner + C in f16/f32
  ACT : dd = dif^2 (Square), s = Abs_reciprocal_sqrt(r2), R0 = c5*s — all
        three functions live in ONE table set (abs_reciprocal_sqrt_and_small),
        preloaded by a warmup rsqrt so exactly one ACT_TABLE_LOAD happens
        (the ln/exp formulation thrashed 9 loads)
  POOL: r2 partial sum, force multiply fv = dif*C, dpdt assembly
  SP  : one in-DMA (q, f32) + one out-DMA (dpdt, f16) per chunk
Raw bass (no TileContext): waits/incs are folded into the instructions, so
the whole module is 75 instructions vs 168 for the tile version.  I/O is
halved vs the full-z formulation: device reads q only (1.18 MB/core) and
writes dpdt only (0.59 MB/core, f16; host upcasts and negates body 2).
CoreSim one-shot: 18.2 us (baseline kernel: 35.9 us sim / 141.5 us measured).
"""
import numpy as np

from concourse import bacc, mybir
from concourse.bass_utils import run_bass_kernel_spmd

F32 = mybir.dt.float32
F16 = mybir.dt.float16
EPS2 = 0.01
SLO = float(1.0 / np.sqrt(130.0))
SHI = float(1.0 / np.sqrt(EPS2))
NCORES = 8
DEG = 5
P = 128
OUT_DT = F16
TLIST = (128, 128)   # per-core chunk sizes; sum(TLIST)*P == B_core


def _silu(x):
    return x / (1.0 + np.exp(-x))


def _dsilu(x):
    sg = 1.0 / (1.0 + np.exp(-x))
    return sg * (1.0 + x * (1.0 - sg))


def _fit_force_poly(W1, b1, W2, b2, W3, deg=DEG):
    """Monomial coefficients of P(s) ~ g(s)*s^2 on [SLO, SHI]; the device
    computes C = s*P(s) = g(s)*s^3 (g = d/ds of the scalar MLP)."""
    W1 = np.asarray(W1, np.float64); b1 = np.asarray(b1, np.float64)
    W2 = np.asarray(W2, np.float64); b2 = np.asarray(b2, np.float64)
    W3 = np.asarray(W3, np.float64)

    def g_exact(s):
        s = np.asarray(s, np.float64)[..., None]
        u1 = s * W1[:, 0] + b1
        a1 = _silu(u1)
        u2 = a1 @ W2.T + b2
        d2 = W3[0] * _dsilu(u2)
        d1 = (d2 @ W2) * _dsilu(u1)
        return d1 @ W1[:, 0]

    n = 4000
    xk = np.cos(np.pi * (np.arange(n) + 0.5) / n)
    ss = SLO + (xk + 1) * (SHI - SLO) / 2
    h = g_exact(ss) * ss * ss
    c = np.polynomial.chebyshev.chebfit(xk, h, deg)
    ch = np.polynomial.chebyshev.Chebyshev(c, domain=[SLO, SHI])
    return np.asarray(ch.convert(kind=np.polynomial.Polynomial).coef,
                      np.float64)


def _build(B_core, coef, tlist=TLIST, deg=DEG):
    coef = [float(c) for c in coef]
    assert len(coef) == deg + 1
    tlist = list(tlist)
    assert sum(tlist) * P == B_core
    n_chunks = len(tlist)
    # stage-major emission keeps every chunk's tiles live: one buffer set
    # per chunk (~23 KiB each vs 208 KiB/partition available)
    nbuf = n_chunks

    nc = bacc.Bacc("TRN2", target_bir_lowering=False, debug=False,
                   num_devices=NCORES)
    q = nc.dram_tensor("q", [B_core, 9], F32, kind="ExternalInput")
    out = nc.dram_tensor("out", [B_core, 9], OUT_DT, kind="ExternalOutput")

    starts = [0]
    for t_ in tlist:
        starts.append(starts[-1] + P * t_)

    bufs = []
    for b in range(nbuf):
        T = max(tlist)
        bufs.append(dict(
            qt=nc.alloc_sbuf_tensor(f"qt{b}", [P, T * 9], F32),
            ot=nc.alloc_sbuf_tensor(f"ot{b}", [P, T * 9], F16),
            dif=nc.alloc_sbuf_tensor(f"dif{b}", [P, T * 9], F32),
            dd=nc.alloc_sbuf_tensor(f"dd{b}", [P, T * 9], F32),
            fv=nc.alloc_sbuf_tensor(f"fv{b}", [P, T * 9], F16),
            r2=nc.alloc_sbuf_tensor(f"r2{b}", [P, T * 3], F32),
            s=nc.alloc_sbuf_tensor(f"s{b}", [P, T * 3], F16),
            R=nc.alloc_sbuf_tensor(f"R{b}", [P, T * 3], F16),
            C=nc.alloc_sbuf_tensor(f"C{b}", [P, T * 3], F32),
        ))

    warm_t = nc.alloc_sbuf_tensor("warm", [P, 1], F32)

    qs = [nc.alloc_semaphore(f"qs{i}") for i in range(n_chunks)]
    vs = [nc.alloc_semaphore(f"vs{i}") for i in range(n_chunks)]
    as_ = [nc.alloc_semaphore(f"as{i}") for i in range(n_chunks)]
    ps = [nc.alloc_semaphore(f"ps{i}") for i in range(n_chunks)]
    os_ = nc.alloc_semaphore("osem")
    ws = nc.alloc_semaphore("wsem")

    AF = mybir.ActivationFunctionType
    ALU = mybir.AluOpType

    with nc.Block() as blk:

        @blk.sync
        def _(sp):
            for ci, T in enumerate(tlist):
                b = bufs[ci % nbuf]
                qch = q[:][starts[ci]:starts[ci + 1], :].rearrange(
                    "(p t) f -> p (t f)", p=P, t=T)
                if ci >= nbuf:
                    sp.wait_ge(os_, 16 * (ci - nbuf + 1))
                sp.dma_start(b["qt"][:, :T * 9], qch).then_inc(qs[ci], 16)
            for ci, T in enumerate(tlist):
                b = bufs[ci % nbuf]
                och = out[:][starts[ci]:starts[ci + 1], :].rearrange(
                    "(p t) f -> p (t f)", p=P, t=T)
                sp.wait_ge(ps[ci], 2)   # r2 partial + assembly both done
                sp.dma_start(och, b["ot"][:, :T * 9]).then_inc(os_, 16)
            sp.wait_ge(os_, 16 * n_chunks)

        @blk.vector
        def _(v):
            # stage-major: cross-chunk lookahead hides ACT/POOL latency
            for ci, T in enumerate(tlist):
                b = bufs[ci % nbuf]
                qf = b["qt"][:, :T * 9].rearrange("p (t f) -> p f t", f=9)
                difv = b["dif"][:, :T * 9].rearrange("p (kc t) -> p kc t",
                                                     kc=9)
                v.wait_ge(qs[ci], 16)
                # pair order k0=(0,1), k1=(1,2), k2=(0,2): first two pairs
                # fused as [q0 q1] - [q1 q2]
                v.tensor_sub(difv[:, 0:6, :], qf[:, 0:6, :], qf[:, 3:9, :])
                v.tensor_sub(difv[:, 6:9, :], qf[:, 0:3, :],
                             qf[:, 6:9, :]).then_inc(vs[ci], 1)
            for ci, T in enumerate(tlist):
                b = bufs[ci % nbuf]
                dd4 = b["dd"][:, :T * 9].rearrange("p (k c t) -> p k c t",
                                                   k=3, c=3)
                r2v = b["r2"][:, :T * 3].rearrange("p (k t) -> p k t", k=3)
                v.wait_ge(ps[ci], 1)    # pool partial sum dd0+dd1 ready
                v.scalar_tensor_tensor(r2v[:, :, :], r2v[:, :, :], EPS2,
                                       dd4[:, :, 2, :],
                                       ALU.add, ALU.add).then_inc(vs[ci], 1)
            for ci, T in enumerate(tlist):
                b = bufs[ci % nbuf]
                sT = b["s"][:, :T * 3]
                RT = b["R"][:, :T * 3]
                CT = b["C"][:, :T * 3]
                v.wait_ge(as_[ci], 3)   # s + R0 ready
                for k in range(deg - 1, 0, -1):
                    v.scalar_tensor_tensor(RT[:], RT[:], coef[k], sT[:],
                                           ALU.add, ALU.mult)
                v.scalar_tensor_tensor(CT[:], RT[:], coef[0], sT[:],
                                       ALU.add, ALU.mult).then_inc(vs[ci], 1)

        @blk.scalar
        def _(a):
            # warmup rsqrt on a pool-memset scratch loads the single ACT
            # table set before the first Square, off the critical path
            a.wait_ge(ws, 1)
            a.activation(warm_t[:], warm_t[:], AF.Abs_reciprocal_sqrt)
            for ci, T in enumerate(tlist):
                b = bufs[ci % nbuf]
                a.wait_ge(vs[ci], 1)   # dif ready
                a.activation(b["dd"][:, :T * 9], b["dif"][:, :T * 9],
                             AF.Square).then_inc(as_[ci], 1)
            for ci, T in enumerate(tlist):
                b = bufs[ci % nbuf]
                a.wait_ge(vs[ci], 2)   # r2 ready
                a.activation(b["s"][:, :T * 3], b["r2"][:, :T * 3],
                             AF.Abs_reciprocal_sqrt).then_inc(as_[ci], 1)
                a.activation(b["R"][:, :T * 3], b["s"][:, :T * 3], AF.Copy,
                             scale=coef[deg]).then_inc(as_[ci], 1)

        @blk.gpsimd
        def _(g):
            g.memset(warm_t[:], 1.0).then_inc(ws, 1)
            for ci, T in enumerate(tlist):
                b = bufs[ci % nbuf]
                dd4 = b["dd"][:, :T * 9].rearrange("p (k c t) -> p k c t",
                                                   k=3, c=3)
                r2v = b["r2"][:, :T * 3].rearrange("p (k t) -> p k t", k=3)
                g.wait_ge(as_[ci], 1)   # dd ready
                g.tensor_add(r2v[:, :, :], dd4[:, :, 0, :],
                             dd4[:, :, 1, :]).then_inc(ps[ci], 1)
            for ci, T in enumerate(tlist):
                b = bufs[ci % nbuf]
                dif4 = b["dif"][:, :T * 9].rearrange("p (k c t) -> p k c t",
                                                     k=3, c=3)
                fv4 = b["fv"][:, :T * 9].rearrange("p (k c t) -> p k c t",
                                                   k=3, c=3)
                ov = b["ot"][:, :T * 9].rearrange("p (t f) -> p f t", f=9)
                CT = b["C"][:, :T * 3]
                Cb = CT.rearrange("p (k one t) -> p k one t", k=3, one=1)
                Cb = Cb.broadcast_to([P, 3, 3, T])
                g.wait_ge(vs[ci], 3)   # dif + C ready
                g.tensor_mul(fv4[:, :, :, :], dif4[:, :, :, :], Cb)
                # body2 emitted positive; host negates during upcast
                # (TensorScalarPtr is illegal on Pool, so all three are TT)
                g.tensor_add(ov[:, 0:3, :], fv4[:, 0, :, :], fv4[:, 2, :, :])
                g.tensor_sub(ov[:, 3:6, :], fv4[:, 1, :, :], fv4[:, 0, :, :])
                g.tensor_add(ov[:, 6:9, :], fv4[:, 1, :, :],
                             fv4[:, 2, :, :]).then_inc(ps[ci], 1)

    nc.compile()
    return nc


_MODULE_CACHE = {}


def _get_module(B_core, coef, tlist=TLIST, deg=DEG):
    coef32 = np.asarray(coef, np.float32)
    key = (B_core, tuple(tlist), deg, coef32.tobytes())
    if key not in _MODULE_CACHE:
        _MODULE_CACHE[key] = _build(B_core, coef32, tlist, deg)
    return _MODULE_CACHE[key]


def kernel(z, log_m_body, W1, b1, W2, b2, W3, b3, **_unused):
    z = np.asarray(z, np.float32)
    B = z.shape[0]

    coef = _fit_force_poly(W1, b1, W2, b2, W3).astype(np.float32)
    minv = (np.float32(1.0)
            / (np.exp(np.asarray(log_m_body, np.float32)) + np.float32(1e-8)))

    rows_core = P * sum(TLIST)
    grain = NCORES * rows_core
    B_pad = ((B + grain - 1) // grain) * grain
    q = z[:, 0:9]
    if B_pad != B:
        qp = np.zeros((B_pad, 9), np.float32)
        qp[:B] = q
    else:
        qp = np.ascontiguousarray(q)
    B_core = B_pad // NCORES

    tlist = TLIST * (B_core // rows_core)   # repeat chunk pattern to cover B
    nc = _get_module(B_core, coef, tlist=tlist)
    in_maps = [
        {"q": np.ascontiguousarray(qp[i * B_core:(i + 1) * B_core])}
        for i in range(NCORES)
    ]
    res = run_bass_kernel_spmd(nc, in_maps, core_ids=list(range(NCORES)))
    dpdt = np.concatenate([r["out"] for r in res.results], axis=0)[:B]

    out = np.empty((B, 18), np.float32)
    # dqdt = p * minv: trivial affine map of the input, host side
    out[:, 0:9] = z[:, 9:18] * np.repeat(minv, 3)[None, :]
    out[:, 9:15] = dpdt[:, 0:6].astype(np.float32)
    out[:, 15:18] = -dpdt[:, 6:9].astype(np.float32)
    return out


# revision 7
# speedup vs baseline: 1.0276x; 1.0276x over previous
"""Trainium2 Bass kernel for the 3-body Hamiltonian-NN time-derivative.

out = J grad_z H(z): dqdt = p * minv (trivial affine map, assembled on the
host); dpdt = pairwise forces, computed on device from q only.

The potential's tiny MLP acts on the scalar pairwise inverse distance
s = 1/sqrt(r2+eps2), so its gradient contribution reduces to a smooth 1-D
coefficient C(s) = g(s)*s^3 multiplying each pair-difference vector.  C is
distilled at runtime from the MLP weights into a degree-5 polynomial in s
(Chebyshev fit on s in [1/sqrt(130), 10]; end-to-end rel err 9.1e-3 vs the
2e-2 gate, f16 Horner included).

Device pipeline per 128-row chunk (2 chunks of T=128 per core, pure data
parallel over 8 cores):
  DVE : pair diffs (f32, keeps cancellation digits), r2 eps-fold STT,
        5-step Horner + C in f16/f32
  ACT : dd = dif^2 (Square), s = Abs_reciprocal_sqrt(r2), R0 = c5*s — all
        three functions live in ONE table set (abs_reciprocal_sqrt_and_small),
        preloaded by a warmup rsqrt so exactly one ACT_TABLE_LOAD happens
        (the ln/exp formulation thrashed 9 loads)
  POOL: r2 partial sum, force multiply fv = dif*C, dpdt assembly
  SP  : one in-DMA (q, f32) + one out-DMA (dpdt, f16) per chunk
Raw bass (no TileContext): waits/incs are folded into the instructions, so
the whole module is 75 instructions vs 168 for the tile version.  I/O is
halved vs the full-z formulation: device reads q only (1.18 MB/core) and
writes dpdt only (0.59 MB/core, f16; host upcasts and negates body 2).
CoreSim one-shot: 18.2 us (baseline kernel: 35.9 us sim / 141.5 us measured).
"""
import numpy as np

from concourse import bacc, mybir
from concourse.bass_utils import run_bass_kernel_spmd

F32 = mybir.dt.float32
F16 = mybir.dt.float16
EPS2 = 0.01
SLO = float(1.0 / np.sqrt(130.0))
SHI = float(1.0 / np.sqrt(EPS2))
NCORES = 8
DEG = 5
P = 128
OUT_DT = F16
TLIST = (128, 128)   # per-core chunk sizes; sum(TLIST)*P == B_core


def _silu(x):
    return x / (1.0 + np.exp(-x))


def _dsilu(x):
    sg = 1.0 / (1.0 + np.exp(-x))
    return sg * (1.0 + x * (1.0 - sg))


def _fit_force_poly(W1, b1, W2, b2, W3, deg=DEG):
    """Monomial coefficients of P(s) ~ g(s)*s^2 on [SLO, SHI]; the device
    computes C = s*P(s) = g(s)*s^3 (g = d/ds of the scalar MLP)."""
    W1 = np.asarray(W1, np.float64); b1 = np.asarray(b1, np.float64)
    W2 = np.asarray(W2, np.float64); b2 = np.asarray(b2, np.float64)
    W3 = np.asarray(W3, np.float64)

    def g_exact(s):
        s = np.asarray(s, np.float64)[..., None]
        u1 = s * W1[:, 0] + b1
        a1 = _silu(u1)
        u2 = a1 @ W2.T + b2
        d2 = W3[0] * _dsilu(u2)
        d1 = (d2 @ W2) * _dsilu(u1)
        return d1 @ W1[:, 0]

    n = 4000
    xk = np.cos(np.pi * (np.arange(n) + 0.5) / n)
    ss = SLO + (xk + 1) * (SHI - SLO) / 2
    h = g_exact(ss) * ss * ss
    c = np.polynomial.chebyshev.chebfit(xk, h, deg)
    ch = np.polynomial.chebyshev.Chebyshev(c, domain=[SLO, SHI])
    return np.asarray(ch.convert(kind=np.polynomial.Polynomial).coef,
                      np.float64)


def _build(B_core, coef, tlist=TLIST, deg=DEG):
    coef = [float(c) for c in coef]
    assert len(coef) == deg + 1
    tlist = list(tlist)
    assert sum(tlist) * P == B_core
    n_chunks = len(tlist)
    # stage-major emission keeps every chunk's tiles live: one buffer set
    # per chunk (~23 KiB each vs 208 KiB/partition available)
    nbuf = n_chunks

    nc = bacc.Bacc("TRN2", target_bir_lowering=False, debug=False,
                   num_devices=NCORES)
    q = nc.dram_tensor("q", [B_core, 9], F32, kind="ExternalInput")
    out = nc.dram_tensor("out", [B_core, 9], OUT_DT, kind="ExternalOutput")

    starts = [0]
    for t_ in tlist:
        starts.append(starts[-1] + P * t_)

    bufs = []
    for b in range(nbuf):
        T = max(tlist)
        bufs.append(dict(
            qt=nc.alloc_sbuf_tensor(f"qt{b}", [P, T * 9], F32),
            ot=nc.alloc_sbuf_tensor(f"ot{b}", [P, T * 9], F16),
            dif=nc.alloc_sbuf_tensor(f"dif{b}", [P, T * 9], F32),
            dd=nc.alloc_sbuf_tensor(f"dd{b}", [P, T * 9], F32),
            fv=nc.alloc_sbuf_tensor(f"fv{b}", [P, T * 9], F16),
            r2=nc.alloc_sbuf_tensor(f"r2{b}", [P, T * 3], F32),
            s=nc.alloc_sbuf_tensor(f"s{b}", [P, T * 3], F16),
            R=nc.alloc_sbuf_tensor(f"R{b}", [P, T * 3], F16),
            C=nc.alloc_sbuf_tensor(f"C{b}", [P, T * 3], F32),
        ))

    warm_t = nc.alloc_sbuf_tensor("warm", [P, 1], F32)

    qs = [nc.alloc_semaphore(f"qs{i}") for i in range(n_chunks)]
    vs = [nc.alloc_semaphore(f"vs{i}") for i in range(n_chunks)]
    as_ = [nc.alloc_semaphore(f"as{i}") for i in range(n_chunks)]
    ps = [nc.alloc_semaphore(f"ps{i}") for i in range(n_chunks)]
    fs = [nc.alloc_semaphore(f"fs{i}") for i in range(n_chunks)]
    os_ = nc.alloc_semaphore("osem")
    ws = nc.alloc_semaphore("wsem")

    AF = mybir.ActivationFunctionType
    ALU = mybir.AluOpType

    with nc.Block() as blk:

        @blk.sync
        def _(sp):
            for ci, T in enumerate(tlist):
                b = bufs[ci % nbuf]
                qch = q[:][starts[ci]:starts[ci + 1], :].rearrange(
                    "(p t) f -> p (t f)", p=P, t=T)
                if ci >= nbuf:
                    sp.wait_ge(os_, 16 * (ci - nbuf + 1))
                sp.dma_start(b["qt"][:, :T * 9], qch).then_inc(qs[ci], 16)
            for ci, T in enumerate(tlist):
                b = bufs[ci % nbuf]
                och = out[:][starts[ci]:starts[ci + 1], :].rearrange(
                    "(p t) f -> p (t f)", p=P, t=T)
                sp.wait_ge(ps[ci], 1)   # pool assembly (a0, a2) done
                sp.wait_ge(vs[ci], 4)   # DVE assembly (a1) done
                sp.dma_start(och, b["ot"][:, :T * 9]).then_inc(os_, 16)
            sp.wait_ge(os_, 16 * n_chunks)

        @blk.vector
        def _(v):
            # stage-major: cross-chunk lookahead hides ACT/POOL latency
            for ci, T in enumerate(tlist):
                b = bufs[ci % nbuf]
                qf = b["qt"][:, :T * 9].rearrange("p (t f) -> p f t", f=9)
                difv = b["dif"][:, :T * 9].rearrange("p (kc t) -> p kc t",
                                                     kc=9)
                v.wait_ge(qs[ci], 16)
                # pair order k0=(0,1), k1=(1,2), k2=(0,2): first two pairs
                # fused as [q0 q1] - [q1 q2]
                v.tensor_sub(difv[:, 0:6, :], qf[:, 0:6, :], qf[:, 3:9, :])
                v.tensor_sub(difv[:, 6:9, :], qf[:, 0:3, :],
                             qf[:, 6:9, :]).then_inc(vs[ci], 1)
            for ci, T in enumerate(tlist):
                b = bufs[ci % nbuf]
                dd4 = b["dd"][:, :T * 9].rearrange("p (k c t) -> p k c t",
                                                   k=3, c=3)
                r2v = b["r2"][:, :T * 3].rearrange("p (k t) -> p k t", k=3)
                v.wait_ge(as_[ci], 1)   # dd ready
                v.tensor_add(r2v[:, :, :], dd4[:, :, 0, :], dd4[:, :, 1, :])
                v.scalar_tensor_tensor(r2v[:, :, :], r2v[:, :, :], EPS2,
                                       dd4[:, :, 2, :],
                                       ALU.add, ALU.add).then_inc(vs[ci], 1)
            for ci, T in enumerate(tlist):
                b = bufs[ci % nbuf]
                sT = b["s"][:, :T * 3]
                RT = b["R"][:, :T * 3]
                CT = b["C"][:, :T * 3]
                v.wait_ge(as_[ci], 3)   # s + R0 ready
                for k in range(deg - 1, 0, -1):
                    v.scalar_tensor_tensor(RT[:], RT[:], coef[k], sT[:],
                                           ALU.add, ALU.mult)
                v.scalar_tensor_tensor(CT[:], RT[:], coef[0], sT[:],
                                       ALU.add, ALU.mult).then_inc(vs[ci], 1)
            for ci, T in enumerate(tlist):
                b = bufs[ci % nbuf]
                fv4 = b["fv"][:, :T * 9].rearrange("p (k c t) -> p k c t",
                                                   k=3, c=3)
                ov = b["ot"][:, :T * 9].rearrange("p (t f) -> p f t", f=9)
                v.wait_ge(fs[ci], 1)    # fv ready (pool)
                v.tensor_sub(ov[:, 3:6, :], fv4[:, 1, :, :],
                             fv4[:, 0, :, :]).then_inc(vs[ci], 1)

        @blk.scalar
        def _(a):
            # warmup rsqrt on a pool-memset scratch loads the single ACT
            # table set before the first Square, off the critical path
            a.wait_ge(ws, 1)
            a.activation(warm_t[:], warm_t[:], AF.Abs_reciprocal_sqrt)
            for ci, T in enumerate(tlist):
                b = bufs[ci % nbuf]
                a.wait_ge(vs[ci], 1)   # dif ready
                a.activation(b["dd"][:, :T * 9], b["dif"][:, :T * 9],
                             AF.Square).then_inc(as_[ci], 1)
            for ci, T in enumerate(tlist):
                b = bufs[ci % nbuf]
                a.wait_ge(vs[ci], 2)   # r2 ready
                a.activation(b["s"][:, :T * 3], b["r2"][:, :T * 3],
                             AF.Abs_reciprocal_sqrt).then_inc(as_[ci], 1)
                a.activation(b["R"][:, :T * 3], b["s"][:, :T * 3], AF.Copy,
                             scale=coef[deg]).then_inc(as_[ci], 1)

        @blk.gpsimd
        def _(g):
            # Pool runs Add/Multiply at ~0.42 of roofline (+95ns Q7 launch) on
            # HW, so it carries only the force multiply and two of the three
            # assembly adds; r2 partial and the third assembly stay on DVE
            g.memset(warm_t[:], 1.0).then_inc(ws, 1)
            for ci, T in enumerate(tlist):
                b = bufs[ci % nbuf]
                dif4 = b["dif"][:, :T * 9].rearrange("p (k c t) -> p k c t",
                                                     k=3, c=3)
                fv4 = b["fv"][:, :T * 9].rearrange("p (k c t) -> p k c t",
                                                   k=3, c=3)
                ov = b["ot"][:, :T * 9].rearrange("p (t f) -> p f t", f=9)
                CT = b["C"][:, :T * 3]
                Cb = CT.rearrange("p (k one t) -> p k one t", k=3, one=1)
                Cb = Cb.broadcast_to([P, 3, 3, T])
                g.wait_ge(vs[ci], 3)   # dif + C ready
                g.tensor_mul(fv4[:, :, :, :], dif4[:, :, :, :],
                             Cb).then_inc(fs[ci], 1)
                # body2 emitted positive; host negates during upcast
                # (TensorScalarPtr is illegal on Pool, so all ops are TT)
                g.tensor_add(ov[:, 0:3, :], fv4[:, 0, :, :], fv4[:, 2, :, :])
                g.tensor_add(ov[:, 6:9, :], fv4[:, 1, :, :],
                             fv4[:, 2, :, :]).then_inc(ps[ci], 1)

    nc.compile()
    return nc


_MODULE_CACHE = {}


def _get_module(B_core, coef, tlist=TLIST, deg=DEG):
    coef32 = np.asarray(coef, np.float32)
    key = (B_core, tuple(tlist), deg, coef32.tobytes())
    if key not in _MODULE_CACHE:
        _MODULE_CACHE[key] = _build(B_core, coef32, tlist, deg)
    return _MODULE_CACHE[key]


def kernel(z, log_m_body, W1, b1, W2, b2, W3, b3, **_unused):
    z = np.asarray(z, np.float32)
    B = z.shape[0]

    coef = _fit_force_poly(W1, b1, W2, b2, W3).astype(np.float32)
    minv = (np.float32(1.0)
            / (np.exp(np.asarray(log_m_body, np.float32)) + np.float32(1e-8)))

    rows_core = P * sum(TLIST)
    grain = NCORES * rows_core
    B_pad = ((B + grain - 1) // grain) * grain
    q = z[:, 0:9]
    if B_pad != B:
        qp = np.zeros((B_pad, 9), np.float32)
        qp[:B] = q
    else:
        qp = np.ascontiguousarray(q)
    B_core = B_pad // NCORES

    tlist = TLIST * (B_core // rows_core)   # repeat chunk pattern to cover B
    nc = _get_module(B_core, coef, tlist=tlist)
    in_maps = [
        {"q": np.ascontiguousarray(qp[i * B_core:(i + 1) * B_core])}
        for i in range(NCORES)
    ]
    res = run_bass_kernel_spmd(nc, in_maps, core_ids=list(range(NCORES)))
    dpdt = np.concatenate([r["out"] for r in res.results], axis=0)[:B]

    out = np.empty((B, 18), np.float32)
    # dqdt = p * minv: trivial affine map of the input, host side
    out[:, 0:9] = z[:, 9:18] * np.repeat(minv, 3)[None, :]
    out[:, 9:15] = dpdt[:, 0:6].astype(np.float32)
    out[:, 15:18] = -dpdt[:, 6:9].astype(np.float32)
    return out
